# revision 38
# baseline (speedup 1.0000x reference)
"""AttentionBlock (GroupNorm + 8-head attention + proj + residual) on 8 TRN2 NeuronCores.

Data-parallel over batch: 16 batches -> 2 per core, no collectives.
Per-core kernel layout (channels on partitions, spatial T=1024 on free dim):
  - GroupNorm(32, 512): bn_stats per channel, cross-partition 16-channel group
    aggregation via tiny PE matmuls (selector/expander matrices), in-place
    normalize as x*A + B with per-channel A/B.
  - qkv: fp32r matmuls. Q,K produced as [c, t]; V produced pre-transposed
    [s, c] directly (hnorm as stationary operand). Biases folded into PSUM
    evictions. Attention scale (1/8) folded into Q weights host-side.
  - attention per (batch, head): S^T = K^T Q in PSUM -> exp on ScalarE during
    eviction -> AV with M=65 (ones column appended to V^T gives softmax
    denominators for free) -> reciprocal + gpsimd partition_broadcast ->
    fused normalize-evict into a_norm.
  - proj: fp32r matmuls; bias + residual fused in one scalar_tensor_tensor
    eviction.
"""

import numpy as np
from contextlib import ExitStack

import concourse.bass as bass
import concourse.tile as tile
from concourse import mybir
from concourse.bass_utils import run_bass_kernel_spmd

B, C, T = 16, 512, 1024
NH, CH = 8, 64
GS = 16  # channels per GroupNorm group
EPS = 1e-5
NCORES = 8
BL = B // NCORES  # batches per core
P = 128
F32 = mybir.dt.float32
F32R = mybir.dt.float32r
BF16 = mybir.dt.bfloat16
AF = mybir.ActivationFunctionType
OP = mybir.AluOpType

VT_W = 65  # per-head V^T columns: 64 channels + 1 ones column


def _kernel_body(nc, tc, ap, out_ap):
    ctx = tc._ctx  # ExitStack attached by _build

    const = ctx.enter_context(tc.tile_pool(name="const", bufs=1))
    gn_pool = ctx.enter_context(tc.tile_pool(name="gn", bufs=2))
    qk_pool = ctx.enter_context(tc.tile_pool(name="qk", bufs=1))
    ew_pool = ctx.enter_context(tc.tile_pool(name="ew", bufs=4))
    rc_pool = ctx.enter_context(tc.tile_pool(name="rc", bufs=4))
    outp = ctx.enter_context(tc.tile_pool(name="outp", bufs=4))
    psA = ctx.enter_context(tc.tile_pool(name="psA", bufs=2, space="PSUM"))
    psC = ctx.enter_context(tc.tile_pool(name="psC", bufs=4, space="PSUM"))

    xv = ap["x"].rearrange("b (j p) t -> b j p t", p=P)  # [BL, 4, 128, T]
    ov = out_ap.rearrange("b (j p) t -> b j p t", p=P)

    # ---------------- x load first (GroupNorm needs it immediately) ----------------
    xf = const.tile([P, BL, 4, T], BF16)  # raw x (bf16 copy for GN/h)
    xh = const.tile([P, BL, 4, T], BF16)  # normalized h (bf16)
    xvr = ap["xbf"].rearrange("b (j p) t -> b j p t", p=P)
    for b in range(BL):
        for j in range(4):
            nc.sync.dma_start(out=xf[:, b, j, :], in_=xvr[b, j])

    # ---------------- constants ----------------
    wq_sb = const.tile([P, 4, 3 * C], BF16)  # w_qkv^T: [cin_part, cin_tile, out]
    wqv = ap["wqkvT"].rearrange("(j p) o -> p j o", p=P)
    for j in range(4):
        nc.sync.dma_start(out=wq_sb[:, j, :], in_=wqv[:, j, :])
    wp_sb = const.tile([P, 4, C], BF16)  # w_proj^T
    nc.sync.dma_start(out=wp_sb, in_=ap["wprojT"].rearrange("(j p) o -> p j o", p=P))
    bqk_sb = const.tile([P, 8], F32)
    nc.sync.dma_start(out=bqk_sb, in_=ap["bqk"])
    bp_sb = const.tile([P, 4], F32)
    nc.sync.dma_start(out=bp_sb, in_=ap["bp"])
    gscale_sb = const.tile([P, 4], F32)
    nc.sync.dma_start(out=gscale_sb, in_=ap["gscale"])
    gbias_sb = const.tile([P, 4], F32)
    nc.sync.dma_start(out=gbias_sb, in_=ap["gbias"])
    gsel_sb = const.tile([P, 8], F32)
    nc.sync.dma_start(out=gsel_sb, in_=ap["gsel"])
    gexp_sb = const.tile([8, P], F32)
    nc.sync.dma_start(out=gexp_sb, in_=ap["gexp"])
    bv_sb = const.tile([P, C], F32)  # V bias broadcast across partitions
    nc.sync.dma_start(out=bv_sb, in_=ap["bv"].partition_broadcast(P))
    eps_sb = const.tile([8, 1], F32)
    nc.vector.memset(eps_sb, EPS)
    expwarm = const.tile([1, 1], F32)
    nc.scalar.activation(out=expwarm, in_=eps_sb[0:1, :], func=AF.Exp)

    # ---------------- GroupNorm (both batches) ----------------
    for b in range(BL):
        bnraw = gn_pool.tile([P, 4, 2, 6], F32, tag="bnraw")
        mv = gn_pool.tile([P, 4, 2], F32, tag="mv")
        for j in range(4):
            for hf in range(2):
                nc.vector.bn_stats(
                    out=bnraw[:, j, hf, :], in_=xf[:, b, j, 512 * hf : 512 * (hf + 1)]
                )
            nc.vector.bn_aggr(out=mv[:, j, :], in_=bnraw[:, j, :, :])
        # m2: cols 0-3 per-channel mean (per c-tile), cols 4-7 per-channel E[x^2]
        m2 = gn_pool.tile([P, 8], F32, tag="m2")
        nc.vector.tensor_copy(out=m2[:, 0:4], in_=mv[:, :, 0])
        nc.vector.tensor_mul(out=m2[:, 4:8], in0=mv[:, :, 0], in1=mv[:, :, 0])
        nc.vector.tensor_add(out=m2[:, 4:8], in0=m2[:, 4:8], in1=mv[:, :, 1])
        # group-aggregate across the 16-channel groups (partition dim) on PE
        gst_ps = psA.tile([P, T], F32, tag="mm")
        nc.tensor.matmul(
            out=gst_ps[0:8, 0:8], lhsT=gsel_sb, rhs=m2, start=True, stop=True
        )
        gs = gn_pool.tile([8, 8], F32, tag="gs")  # cols 0-3 mu_g, 4-7 E2_g
        nc.vector.tensor_scalar_mul(out=gs, in0=gst_ps[0:8, 0:8], scalar1=1.0 / GS)
        musq = gn_pool.tile([8, 4], F32, tag="musq")
        nc.vector.tensor_mul(out=musq, in0=gs[:, 0:4], in1=gs[:, 0:4])
        var = gn_pool.tile([8, 4], F32, tag="var")
        nc.vector.tensor_sub(out=var, in0=gs[:, 4:8], in1=musq)
        # rstd = 1/sqrt(var+eps), one Newton-Raphson refinement for accuracy
        sq = gn_pool.tile([8, 4], F32, tag="sq")
        nc.scalar.activation(out=sq, in_=var, func=AF.Sqrt, bias=eps_sb)
        y0 = gn_pool.tile([8, 4], F32, tag="y0")
        nc.vector.reciprocal(out=y0, in_=sq)
        t1 = gn_pool.tile([8, 4], F32, tag="t1")
        nc.vector.tensor_mul(out=t1, in0=y0, in1=y0)
        vpe = gn_pool.tile([8, 4], F32, tag="vpe")
        nc.vector.tensor_scalar_add(out=vpe, in0=var, scalar1=EPS)
        nc.vector.tensor_mul(out=t1, in0=t1, in1=vpe)
        nc.vector.tensor_scalar(
            out=t1, in0=t1, scalar1=-0.5, scalar2=1.5, op0=OP.mult, op1=OP.add
        )
        nc.vector.tensor_mul(out=gs[:, 4:8], in0=y0, in1=t1)  # rstd into gs cols 4-7
        # expand group stats back to per-channel on PE
        pc_ps = psA.tile([P, T], F32, tag="mm")
        nc.tensor.matmul(
            out=pc_ps[:, 0:8], lhsT=gexp_sb, rhs=gs, start=True, stop=True
        )
        pc = gn_pool.tile([P, 8], F32, tag="pc")
        nc.vector.tensor_copy(out=pc, in_=pc_ps[:, 0:8])
        at = gn_pool.tile([P, 4], F32, tag="at")  # A = rstd*gamma
        nc.vector.tensor_mul(out=at, in0=pc[:, 4:8], in1=gscale_sb)
        bt = gn_pool.tile([P, 4], F32, tag="bt")  # B = beta - mu*A
        nc.vector.tensor_mul(out=bt, in0=pc[:, 0:4], in1=at)
        nc.vector.tensor_sub(out=bt, in0=gbias_sb, in1=bt)
        for j in range(4):
            nc.scalar.activation(
                out=xh[:, b, j, :],
                in_=xf[:, b, j, :],
                func=AF.Identity,
                bias=bt[:, j : j + 1],
                scale=at[:, j : j + 1],
            )

    # ---------------- qkv for both batches (bf16 attention operands) --------
    q_sb = qk_pool.tile([P, BL, 4, T], BF16, tag="q")
    k_sb = qk_pool.tile([P, BL, 4, T], BF16, tag="k")
    vt_sb = qk_pool.tile([P, BL, 8, NH * VT_W], BF16, tag="vt")
    a_sb = qk_pool.tile([P, BL, 4, T], BF16, tag="a")
    for b in range(BL):
        # --- Q, K: out rows = 8 m-tiles (Q tiles 0-3, K tiles 4-7)
        for m in range(8):
            ps = psA.tile([P, T], F32, tag="mm")
            for j in range(4):
                for n in range(2):
                    nc.tensor.matmul(
                        out=ps[:, 512 * n : 512 * (n + 1)],
                        lhsT=wq_sb[:, j, P * m : P * (m + 1)],
                        rhs=xh[:, b, j, 512 * n : 512 * (n + 1)],
                        start=(j == 0),
                        stop=(j == 3),
                    )
            if m < 4:
                nc.scalar.activation(
                    out=q_sb[:, b, m, :],
                    in_=ps,
                    func=AF.Identity,
                    bias=bqk_sb[:, m : m + 1],
                )
            else:
                nc.scalar.activation(
                    out=k_sb[:, b, m - 4, :],
                    in_=ps,
                    func=AF.Identity,
                    bias=bqk_sb[:, m : m + 1],
                )

        # --- V^T direct: [s, c] with ones column per head
        for s in range(8):
            ones_view = vt_sb[:, b, s, :].rearrange("p (h c) -> p h c", c=VT_W)[
                :, :, CH : CH + 1
            ]
            nc.vector.memset(ones_view, 1.0)
        for s in range(8):
            ps_full = psA.tile([P, T], F32, tag="mm")
            ps = ps_full[:, 0:C]
            for j in range(4):
                nc.tensor.matmul(
                    out=ps,
                    lhsT=xh[:, b, j, P * s : P * (s + 1)],
                    rhs=wq_sb[:, j, 2 * C : 3 * C],
                    start=(j == 0),
                    stop=(j == 3),
                )
            dst = vt_sb[:, b, s, :].rearrange("p (h c) -> p h c", c=VT_W)[:, :, 0:CH]
            nc.vector.tensor_tensor(
                out=dst,
                in0=ps.rearrange("p (h c) -> p h c", c=CH),
                in1=bv_sb.rearrange("p (h c) -> p h c", c=CH),
                op=OP.add,
            )

    # ---------------- attention: heads of both batches interleaved ----------
    for h in range(8):
        for b in range(BL):
            pof = 64 * (h % 2)
            jt = h // 2
            qh = q_sb[pof : pof + 64, b, jt, :]
            kh = k_sb[pof : pof + 64, b, jt, :]
            accs = []
            for n in range(2):
                acc = psC.tile([VT_W, 512], F32, tag="av")
                accs.append(acc)
            for s in range(8):
                sps = psA.tile([P, T], F32, tag="mm")
                for n in range(2):
                    nc.tensor.matmul(
                        out=sps[:, 512 * n : 512 * (n + 1)],
                        lhsT=kh[:, P * s : P * (s + 1)],
                        rhs=qh[:, 512 * n : 512 * (n + 1)],
                        start=True,
                        stop=True,
                    )
                ew = ew_pool.tile([P, T], BF16, tag="ew")
                nc.scalar.activation(out=ew, in_=sps, func=AF.Exp)
                for n in range(2):
                    nc.tensor.matmul(
                        out=accs[n],
                        lhsT=vt_sb[:, b, s, VT_W * h : VT_W * (h + 1)],
                        rhs=ew[:, 512 * n : 512 * (n + 1)],
                        start=(s == 0),
                        stop=(s == 7),
                    )
            for n in range(2):
                acc = accs[n]
                nsl = slice(512 * n, 512 * (n + 1))
                # softmax denominators live in row 64 of acc
                rr = rc_pool.tile([VT_W, 512], F32, tag="rr")
                nc.vector.reciprocal(out=rr[64:65, :], in_=acc[64:65, :])
                rz = rc_pool.tile([1, 512], F32, tag="rz")
                nc.gpsimd.dma_start(out=rz, in_=rr[64:65, :])
                rb = rc_pool.tile([64, 512], F32, tag="rb")
                nc.gpsimd.partition_broadcast(out_ap=rb, in_ap=rz, channels=64)
                if h % 2 == 0:
                    nc.vector.tensor_mul(
                        out=a_sb[0:64, b, jt, nsl], in0=acc[0:64, :], in1=rb
                    )
                else:
                    stg = rc_pool.tile([64, 512], BF16, tag="stg")
                    nc.vector.tensor_mul(out=stg, in0=acc[0:64, :], in1=rb)
                    nc.gpsimd.dma_start(out=a_sb[64:128, b, jt, nsl], in_=stg)

    # ---------------- proj + bias + residual ----------------
    for b in range(BL):
        xr_tiles = []
        for m in range(4):
            xr_t = outp.tile([P, T], F32, tag="xr")
            nc.sync.dma_start(out=xr_t, in_=xv[b, m])
            xr_tiles.append(xr_t)
        for m in range(4):
            pps = psA.tile([P, T], F32, tag="mm")
            for j in range(4):
                for n in range(2):
                    nc.tensor.matmul(
                        out=pps[:, 512 * n : 512 * (n + 1)],
                        lhsT=wp_sb[:, j, P * m : P * (m + 1)],
                        rhs=a_sb[:, b, j, 512 * n : 512 * (n + 1)],
                        start=(j == 0),
                        stop=(j == 3),
                    )
            xr_t = xr_tiles[m]
            o_t = outp.tile([P, T], F32, tag="o")
            for half in range(2):
                hsl = slice(512 * half, 512 * (half + 1))
                nc.vector.scalar_tensor_tensor(
                    out=o_t[:, hsl],
                    in0=pps[:, hsl],
                    scalar=bp_sb[:, m : m + 1],
                    in1=xr_t[:, hsl],
                    op0=OP.add,
                    op1=OP.add,
                )
                nc.sync.dma_start(out=ov[b, m][:, hsl], in_=o_t[:, hsl])


def build(num_devices=NCORES, debug=False):
    from concourse import bacc

    nc = bacc.Bacc(
        "TRN2", target_bir_lowering=False, debug=debug, num_devices=num_devices
    )
    ap = {}

    def inp(name, shape):
        ap[name] = nc.dram_tensor(name, shape, F32, kind="ExternalInput").ap()

    inp("x", [BL, C, T])
    ap["xbf"] = nc.dram_tensor("xbf", [BL, C, T], BF16, kind="ExternalInput").ap()
    ap["wqkvT"] = nc.dram_tensor("wqkvT", [C, 3 * C], BF16, kind="ExternalInput").ap()
    ap["wprojT"] = nc.dram_tensor("wprojT", [C, C], BF16, kind="ExternalInput").ap()
    inp("bqk", [P, 8])
    inp("bv", [C])
    inp("bp", [P, 4])
    inp("gscale", [P, 4])
    inp("gbias", [P, 4])
    inp("gsel", [P, 8])
    inp("gexp", [8, P])
    out_ap = nc.dram_tensor("out", [BL, C, T], F32, kind="ExternalOutput").ap()

    with tile.TileContext(nc) as tc:
        with ExitStack() as ctx:
            tc._ctx = ctx
            _kernel_body(nc, tc, ap, out_ap)
    nc.compile()
    return nc


def host_prep(x, gn_scale, gn_bias, w_qkv, b_qkv, w_proj, b_proj):
    """Build the shared (weight) input arrays and the full [16,512,1024] x."""
    xr = np.ascontiguousarray(np.asarray(x, np.float32).reshape(B, C, T))
    w_qkv = np.asarray(w_qkv, np.float32)
    b_qkv = np.asarray(b_qkv, np.float32)
    # permute interleaved [head, (q,k,v), ch] rows -> [(q,k,v), head, ch]
    perm = np.array(
        [h * 3 * CH + w * CH + c for w in range(3) for h in range(NH) for c in range(CH)],
        dtype=np.int64,
    )
    wq_p = w_qkv[perm].copy()
    bq_p = b_qkv[perm].copy()
    wq_p[:C] *= 0.125  # attention scale (1/sqrt(sqrt(ch)))^2 folded into Q
    bq_p[:C] *= 0.125
    import ml_dtypes

    shared = {
        "wqkvT": np.ascontiguousarray(wq_p.T).astype(ml_dtypes.bfloat16),
        "wprojT": np.ascontiguousarray(np.asarray(w_proj, np.float32).T).astype(
            ml_dtypes.bfloat16
        ),
        "bqk": np.ascontiguousarray(bq_p[: 2 * C].reshape(8, P).T),
        "bv": np.ascontiguousarray(bq_p[2 * C :]),
        "bp": np.ascontiguousarray(np.asarray(b_proj, np.float32).reshape(4, P).T),
        "gscale": np.ascontiguousarray(
            np.asarray(gn_scale, np.float32).reshape(4, P).T
        ),
        "gbias": np.ascontiguousarray(np.asarray(gn_bias, np.float32).reshape(4, P).T),
        "gsel": np.ascontiguousarray(
            (np.arange(P)[:, None] // GS == np.arange(8)[None, :]).astype(np.float32)
        ),
        "gexp": np.ascontiguousarray(
            (np.arange(8)[:, None] == np.arange(P)[None, :] // GS).astype(np.float32)
        ),
    }
    return xr, shared


_NC_CACHE = {}


def kernel(x, gn_scale, gn_bias, w_qkv, b_qkv, w_proj, b_proj):
    xr, shared = host_prep(x, gn_scale, gn_bias, w_qkv, b_qkv, w_proj, b_proj)
    if "nc" not in _NC_CACHE:
        _NC_CACHE["nc"] = build()
    nc = _NC_CACHE["nc"]
    import ml_dtypes

    xbf = xr.astype(ml_dtypes.bfloat16)
    in_maps = [
        {
            "x": np.ascontiguousarray(xr[i * BL : (i + 1) * BL]),
            "xbf": np.ascontiguousarray(xbf[i * BL : (i + 1) * BL]),
            **shared,
        }
        for i in range(NCORES)
    ]
    res = run_bass_kernel_spmd(nc, in_maps, core_ids=list(range(NCORES)))
    out = np.concatenate([res.results[i]["out"] for i in range(NCORES)], axis=0)
    return np.ascontiguousarray(out.reshape(B, C, 32, 32).astype(np.float32))


# revision 39
# speedup vs baseline: 1.1081x; 1.1081x over previous
"""AttentionBlock (GroupNorm + 8-head attention + proj + residual) on 8 TRN2 NeuronCores.

Data-parallel over batch: 16 batches -> 2 per core, no collectives.
Per-core kernel layout (channels on partitions, spatial T=1024 on free dim):
  - GroupNorm(32, 512): bn_stats per channel, cross-partition 16-channel group
    aggregation via tiny PE matmuls (selector/expander matrices), in-place
    normalize as x*A + B with per-channel A/B.
  - qkv: fp32r matmuls. Q,K produced as [c, t]; V produced pre-transposed
    [s, c] directly (hnorm as stationary operand). Biases folded into PSUM
    evictions. Attention scale (1/8) folded into Q weights host-side.
  - attention per (batch, head): S^T = K^T Q in PSUM -> exp on ScalarE during
    eviction -> AV with M=65 (ones column appended to V^T gives softmax
    denominators for free) -> reciprocal + gpsimd partition_broadcast ->
    fused normalize-evict into a_norm.
  - proj: fp32r matmuls; bias + residual fused in one scalar_tensor_tensor
    eviction.
"""

import numpy as np
from contextlib import ExitStack

import concourse.bass as bass
import concourse.tile as tile
from concourse import mybir
from concourse.bass_utils import run_bass_kernel_spmd

B, C, T = 16, 512, 1024
NH, CH = 8, 64
GS = 16  # channels per GroupNorm group
EPS = 1e-5
NCORES = 8
BL = B // NCORES  # batches per core
P = 128
F32 = mybir.dt.float32
F32R = mybir.dt.float32r
BF16 = mybir.dt.bfloat16
AF = mybir.ActivationFunctionType
OP = mybir.AluOpType

VT_W = 65  # per-head V^T columns: 64 channels + 1 ones column


def _kernel_body(nc, tc, ap, out_ap):
    ctx = tc._ctx  # ExitStack attached by _build

    const = ctx.enter_context(tc.tile_pool(name="const", bufs=1))
    gn_pool = ctx.enter_context(tc.tile_pool(name="gn", bufs=2))
    qk_pool = ctx.enter_context(tc.tile_pool(name="qk", bufs=1))
    ew_pool = ctx.enter_context(tc.tile_pool(name="ew", bufs=4))
    rc_pool = ctx.enter_context(tc.tile_pool(name="rc", bufs=4))
    outp = ctx.enter_context(tc.tile_pool(name="outp", bufs=4))
    psA = ctx.enter_context(tc.tile_pool(name="psA", bufs=2, space="PSUM"))
    psC = ctx.enter_context(tc.tile_pool(name="psC", bufs=4, space="PSUM"))

    xv = ap["x"].rearrange("b (j p) t -> b j p t", p=P)  # [BL, 4, 128, T]
    ov = out_ap.rearrange("b (j p) t -> b j p t", p=P)

    # ---------------- x load first (GroupNorm needs it immediately) ----------------
    xf = const.tile([P, BL, 4, T], BF16)  # raw x (bf16 copy for GN/h)
    xh = const.tile([P, BL, 4, T], BF16)  # normalized h (bf16)
    xvr = ap["xbf"].rearrange("b (j p) t -> b j p t", p=P)
    for b in range(BL):
        for j in range(4):
            nc.sync.dma_start(out=xf[:, b, j, :], in_=xvr[b, j])

    # ---------------- constants ----------------
    wq_sb = const.tile([P, 4, 3 * C], BF16)  # w_qkv^T: [cin_part, cin_tile, out]
    wqv = ap["wqkvT"].rearrange("(j p) o -> p j o", p=P)
    for j in range(4):
        nc.sync.dma_start(out=wq_sb[:, j, :], in_=wqv[:, j, :])
    wp_sb = const.tile([P, 4, C], BF16)  # w_proj^T
    nc.sync.dma_start(out=wp_sb, in_=ap["wprojT"].rearrange("(j p) o -> p j o", p=P))
    bqk_sb = const.tile([P, 8], F32)
    nc.sync.dma_start(out=bqk_sb, in_=ap["bqk"])
    bp_sb = const.tile([P, 4], F32)
    nc.sync.dma_start(out=bp_sb, in_=ap["bp"])
    gscale_sb = const.tile([P, 4], F32)
    nc.sync.dma_start(out=gscale_sb, in_=ap["gscale"])
    gbias_sb = const.tile([P, 4], F32)
    nc.sync.dma_start(out=gbias_sb, in_=ap["gbias"])
    gsel_sb = const.tile([P, 8], F32)
    nc.sync.dma_start(out=gsel_sb, in_=ap["gsel"])
    gexp_sb = const.tile([8, P], F32)
    nc.sync.dma_start(out=gexp_sb, in_=ap["gexp"])
    bv_sb = const.tile([P, C], F32)  # V bias broadcast across partitions
    nc.sync.dma_start(out=bv_sb, in_=ap["bv"].partition_broadcast(P))
    eps_sb = const.tile([8, 1], F32)
    nc.vector.memset(eps_sb, EPS)
    expwarm = const.tile([1, 1], F32)
    nc.scalar.activation(out=expwarm, in_=eps_sb[0:1, :], func=AF.Exp)

    # ---------------- GroupNorm (both batches) ----------------
    for b in range(BL):
        bnraw = gn_pool.tile([P, 4, 2, 6], F32, tag="bnraw")
        mv = gn_pool.tile([P, 4, 2], F32, tag="mv")
        for j in range(4):
            for hf in range(2):
                nc.vector.bn_stats(
                    out=bnraw[:, j, hf, :], in_=xf[:, b, j, 512 * hf : 512 * (hf + 1)]
                )
            nc.vector.bn_aggr(out=mv[:, j, :], in_=bnraw[:, j, :, :])
        # m2: cols 0-3 per-channel mean (per c-tile), cols 4-7 per-channel E[x^2]
        m2 = gn_pool.tile([P, 8], F32, tag="m2")
        nc.vector.tensor_copy(out=m2[:, 0:4], in_=mv[:, :, 0])
        nc.vector.tensor_mul(out=m2[:, 4:8], in0=mv[:, :, 0], in1=mv[:, :, 0])
        nc.vector.tensor_add(out=m2[:, 4:8], in0=m2[:, 4:8], in1=mv[:, :, 1])
        # group-aggregate across the 16-channel groups (partition dim) on PE
        gst_ps = psA.tile([P, T], F32, tag="mm")
        nc.tensor.matmul(
            out=gst_ps[0:8, 0:8], lhsT=gsel_sb, rhs=m2, start=True, stop=True
        )
        gs = gn_pool.tile([8, 8], F32, tag="gs")  # cols 0-3 mu_g, 4-7 E2_g
        nc.vector.tensor_scalar_mul(out=gs, in0=gst_ps[0:8, 0:8], scalar1=1.0 / GS)
        musq = gn_pool.tile([8, 4], F32, tag="musq")
        nc.vector.tensor_mul(out=musq, in0=gs[:, 0:4], in1=gs[:, 0:4])
        var = gn_pool.tile([8, 4], F32, tag="var")
        nc.vector.tensor_sub(out=var, in0=gs[:, 4:8], in1=musq)
        # rstd = 1/sqrt(var+eps), one Newton-Raphson refinement for accuracy
        sq = gn_pool.tile([8, 4], F32, tag="sq")
        nc.scalar.activation(out=sq, in_=var, func=AF.Sqrt, bias=eps_sb)
        y0 = gn_pool.tile([8, 4], F32, tag="y0")
        nc.vector.reciprocal(out=y0, in_=sq)
        t1 = gn_pool.tile([8, 4], F32, tag="t1")
        nc.vector.tensor_mul(out=t1, in0=y0, in1=y0)
        vpe = gn_pool.tile([8, 4], F32, tag="vpe")
        nc.vector.tensor_scalar_add(out=vpe, in0=var, scalar1=EPS)
        nc.vector.tensor_mul(out=t1, in0=t1, in1=vpe)
        nc.vector.tensor_scalar(
            out=t1, in0=t1, scalar1=-0.5, scalar2=1.5, op0=OP.mult, op1=OP.add
        )
        nc.vector.tensor_mul(out=gs[:, 4:8], in0=y0, in1=t1)  # rstd into gs cols 4-7
        # expand group stats back to per-channel on PE
        pc_ps = psA.tile([P, T], F32, tag="mm")
        nc.tensor.matmul(
            out=pc_ps[:, 0:8], lhsT=gexp_sb, rhs=gs, start=True, stop=True
        )
        pc = gn_pool.tile([P, 8], F32, tag="pc")
        nc.vector.tensor_copy(out=pc, in_=pc_ps[:, 0:8])
        at = gn_pool.tile([P, 4], F32, tag="at")  # A = rstd*gamma
        nc.vector.tensor_mul(out=at, in0=pc[:, 4:8], in1=gscale_sb)
        bt = gn_pool.tile([P, 4], F32, tag="bt")  # B = beta - mu*A
        nc.vector.tensor_mul(out=bt, in0=pc[:, 0:4], in1=at)
        nc.vector.tensor_sub(out=bt, in0=gbias_sb, in1=bt)
        for j in range(4):
            nc.scalar.activation(
                out=xh[:, b, j, :],
                in_=xf[:, b, j, :],
                func=AF.Identity,
                bias=bt[:, j : j + 1],
                scale=at[:, j : j + 1],
            )

    # ---------------- qkv for both batches (bf16 attention operands) --------
    q_sb = qk_pool.tile([P, BL, 4, T], BF16, tag="q")
    k_sb = qk_pool.tile([P, BL, 4, T], BF16, tag="k")
    vt_sb = qk_pool.tile([P, BL, 8, NH * VT_W], BF16, tag="vt")
    a_sb = qk_pool.tile([P, BL, 4, T], BF16, tag="a")
    for b in range(BL):
        # --- Q, K: out rows = 8 m-tiles (Q tiles 0-3, K tiles 4-7)
        for m in range(8):
            ps = psA.tile([P, T], F32, tag="mm")
            for j in range(4):
                for n in range(2):
                    nc.tensor.matmul(
                        out=ps[:, 512 * n : 512 * (n + 1)],
                        lhsT=wq_sb[:, j, P * m : P * (m + 1)],
                        rhs=xh[:, b, j, 512 * n : 512 * (n + 1)],
                        start=(j == 0),
                        stop=(j == 3),
                    )
            if m < 4:
                nc.scalar.activation(
                    out=q_sb[:, b, m, :],
                    in_=ps,
                    func=AF.Identity,
                    bias=bqk_sb[:, m : m + 1],
                )
            else:
                nc.scalar.activation(
                    out=k_sb[:, b, m - 4, :],
                    in_=ps,
                    func=AF.Identity,
                    bias=bqk_sb[:, m : m + 1],
                )

        # --- V^T direct: [s, c] with ones column per head
        for s in range(8):
            ones_view = vt_sb[:, b, s, :].rearrange("p (h c) -> p h c", c=VT_W)[
                :, :, CH : CH + 1
            ]
            nc.vector.memset(ones_view, 1.0)
        for s in range(8):
            ps_full = psA.tile([P, T], F32, tag="mm")
            ps = ps_full[:, 0:C]
            for j in range(4):
                nc.tensor.matmul(
                    out=ps,
                    lhsT=xh[:, b, j, P * s : P * (s + 1)],
                    rhs=wq_sb[:, j, 2 * C : 3 * C],
                    start=(j == 0),
                    stop=(j == 3),
                )
            dst = vt_sb[:, b, s, :].rearrange("p (h c) -> p h c", c=VT_W)[:, :, 0:CH]
            nc.vector.tensor_tensor(
                out=dst,
                in0=ps.rearrange("p (h c) -> p h c", c=CH),
                in1=bv_sb.rearrange("p (h c) -> p h c", c=CH),
                op=OP.add,
            )

    # ---------------- attention: heads of both batches interleaved ----------
    for h in range(8):
        for b in range(BL):
            pof = 64 * (h % 2)
            jt = h // 2
            qh = q_sb[pof : pof + 64, b, jt, :]
            kh = k_sb[pof : pof + 64, b, jt, :]
            accs = []
            for n in range(2):
                acc = psC.tile([VT_W, 512], F32, tag="av")
                accs.append(acc)
            for s in range(8):
                sps = psA.tile([P, T], F32, tag="mm")
                for n in range(2):
                    nc.tensor.matmul(
                        out=sps[:, 512 * n : 512 * (n + 1)],
                        lhsT=kh[:, P * s : P * (s + 1)],
                        rhs=qh[:, 512 * n : 512 * (n + 1)],
                        start=True,
                        stop=True,
                    )
                ew = ew_pool.tile([P, T], BF16, tag="ew")
                nc.scalar.activation(out=ew, in_=sps, func=AF.Exp)
                for n in range(2):
                    nc.tensor.matmul(
                        out=accs[n],
                        lhsT=vt_sb[:, b, s, VT_W * h : VT_W * (h + 1)],
                        rhs=ew[:, 512 * n : 512 * (n + 1)],
                        start=(s == 0),
                        stop=(s == 7),
                    )
            for n in range(2):
                acc = accs[n]
                nsl = slice(512 * n, 512 * (n + 1))
                # softmax denominators live in row 64 of acc
                rr = rc_pool.tile([VT_W, 512], F32, tag="rr")
                nc.vector.reciprocal(out=rr[64:65, :], in_=acc[64:65, :])
                rz = rc_pool.tile([1, 512], F32, tag="rz")
                nc.sync.dma_start(out=rz, in_=rr[64:65, :])
                rb = rc_pool.tile([64, 512], F32, tag="rb")
                nc.gpsimd.partition_broadcast(out_ap=rb, in_ap=rz, channels=64)
                if h % 2 == 0:
                    nc.vector.tensor_mul(
                        out=a_sb[0:64, b, jt, nsl], in0=acc[0:64, :], in1=rb
                    )
                else:
                    stg = rc_pool.tile([64, 512], BF16, tag="stg")
                    nc.vector.tensor_mul(out=stg, in0=acc[0:64, :], in1=rb)
                    nc.sync.dma_start(out=a_sb[64:128, b, jt, nsl], in_=stg)

    # ---------------- proj + bias + residual ----------------
    for b in range(BL):
        xr_tiles = []
        for m in range(4):
            xr_t = outp.tile([P, T], F32, tag="xr")
            nc.sync.dma_start(out=xr_t, in_=xv[b, m])
            xr_tiles.append(xr_t)
        for m in range(4):
            pps = psA.tile([P, T], F32, tag="mm")
            for j in range(4):
                for n in range(2):
                    nc.tensor.matmul(
                        out=pps[:, 512 * n : 512 * (n + 1)],
                        lhsT=wp_sb[:, j, P * m : P * (m + 1)],
                        rhs=a_sb[:, b, j, 512 * n : 512 * (n + 1)],
                        start=(j == 0),
                        stop=(j == 3),
                    )
            xr_t = xr_tiles[m]
            o_t = outp.tile([P, T], F32, tag="o")
            for half in range(2):
                hsl = slice(512 * half, 512 * (half + 1))
                nc.vector.scalar_tensor_tensor(
                    out=o_t[:, hsl],
                    in0=pps[:, hsl],
                    scalar=bp_sb[:, m : m + 1],
                    in1=xr_t[:, hsl],
                    op0=OP.add,
                    op1=OP.add,
                )
                nc.sync.dma_start(out=ov[b, m][:, hsl], in_=o_t[:, hsl])


def build(num_devices=NCORES, debug=False):
    from concourse import bacc

    nc = bacc.Bacc(
        "TRN2", target_bir_lowering=False, debug=debug, num_devices=num_devices
    )
    ap = {}

    def inp(name, shape):
        ap[name] = nc.dram_tensor(name, shape, F32, kind="ExternalInput").ap()

    inp("x", [BL, C, T])
    ap["xbf"] = nc.dram_tensor("xbf", [BL, C, T], BF16, kind="ExternalInput").ap()
    ap["wqkvT"] = nc.dram_tensor("wqkvT", [C, 3 * C], BF16, kind="ExternalInput").ap()
    ap["wprojT"] = nc.dram_tensor("wprojT", [C, C], BF16, kind="ExternalInput").ap()
    inp("bqk", [P, 8])
    inp("bv", [C])
    inp("bp", [P, 4])
    inp("gscale", [P, 4])
    inp("gbias", [P, 4])
    inp("gsel", [P, 8])
    inp("gexp", [8, P])
    out_ap = nc.dram_tensor("out", [BL, C, T], F32, kind="ExternalOutput").ap()

    with tile.TileContext(nc) as tc:
        with ExitStack() as ctx:
            tc._ctx = ctx
            _kernel_body(nc, tc, ap, out_ap)
    nc.compile()
    return nc


def host_prep(x, gn_scale, gn_bias, w_qkv, b_qkv, w_proj, b_proj):
    """Build the shared (weight) input arrays and the full [16,512,1024] x."""
    xr = np.ascontiguousarray(np.asarray(x, np.float32).reshape(B, C, T))
    w_qkv = np.asarray(w_qkv, np.float32)
    b_qkv = np.asarray(b_qkv, np.float32)
    # permute interleaved [head, (q,k,v), ch] rows -> [(q,k,v), head, ch]
    perm = np.array(
        [h * 3 * CH + w * CH + c for w in range(3) for h in range(NH) for c in range(CH)],
        dtype=np.int64,
    )
    wq_p = w_qkv[perm].copy()
    bq_p = b_qkv[perm].copy()
    wq_p[:C] *= 0.125  # attention scale (1/sqrt(sqrt(ch)))^2 folded into Q
    bq_p[:C] *= 0.125
    import ml_dtypes

    shared = {
        "wqkvT": np.ascontiguousarray(wq_p.T).astype(ml_dtypes.bfloat16),
        "wprojT": np.ascontiguousarray(np.asarray(w_proj, np.float32).T).astype(
            ml_dtypes.bfloat16
        ),
        "bqk": np.ascontiguousarray(bq_p[: 2 * C].reshape(8, P).T),
        "bv": np.ascontiguousarray(bq_p[2 * C :]),
        "bp": np.ascontiguousarray(np.asarray(b_proj, np.float32).reshape(4, P).T),
        "gscale": np.ascontiguousarray(
            np.asarray(gn_scale, np.float32).reshape(4, P).T
        ),
        "gbias": np.ascontiguousarray(np.asarray(gn_bias, np.float32).reshape(4, P).T),
        "gsel": np.ascontiguousarray(
            (np.arange(P)[:, None] // GS == np.arange(8)[None, :]).astype(np.float32)
        ),
        "gexp": np.ascontiguousarray(
            (np.arange(8)[:, None] == np.arange(P)[None, :] // GS).astype(np.float32)
        ),
    }
    return xr, shared


_NC_CACHE = {}


def kernel(x, gn_scale, gn_bias, w_qkv, b_qkv, w_proj, b_proj):
    xr, shared = host_prep(x, gn_scale, gn_bias, w_qkv, b_qkv, w_proj, b_proj)
    if "nc" not in _NC_CACHE:
        _NC_CACHE["nc"] = build()
    nc = _NC_CACHE["nc"]
    import ml_dtypes

    xbf = xr.astype(ml_dtypes.bfloat16)
    in_maps = [
        {
            "x": np.ascontiguousarray(xr[i * BL : (i + 1) * BL]),
            "xbf": np.ascontiguousarray(xbf[i * BL : (i + 1) * BL]),
            **shared,
        }
        for i in range(NCORES)
    ]
    res = run_bass_kernel_spmd(nc, in_maps, core_ids=list(range(NCORES)))
    out = np.concatenate([res.results[i]["out"] for i in range(NCORES)], axis=0)
    return np.ascontiguousarray(out.reshape(B, C, 32, 32).astype(np.float32))
